# revision 1
# baseline (speedup 1.0000x reference)
"""Self-contained Trainium2 Bass kernel: causal self-attention with ALiBi bias.

Reference computation (B=2, T=2048, C=1024, H=16, Dh=64):
    qkv = x @ W_attn.T + b_attn; split into q,k,v heads
    att = softmax(q.k/sqrt(Dh) + slope_h*min(c-r,0), causal)
    y = (att @ v, heads concat) @ W_proj.T + b_proj

Sharding (8 cores): 2-way data parallel on batch x 4-way tensor parallel on
heads (4 heads/core). Each core computes qkv for its heads over its batch,
full TxT attention for those heads, and the partial output projection over
its heads' 256 columns of W_proj; the host sums the 4 partials per batch.

Per-core dataflow keeps the feature dim on partitions ("transposed"):
  x.T via PE transposes -> qkv matmuls emit q.T/k.T/v.T -> scores computed
  as S.T[j,t] tiles (K=65 matmul; row 64 of q'.T carries -slope*(t%1024),
  row 64 of k'.T is ones) -> exp on ACT with per-partition ALiBi bias
  slope*(j - tpair0); the residual per-column exponent terms cancel in
  normalization (row 64 of v' is ones, giving the softmax denominator
  through the same PV matmul) -> P.T tiles feed PV directly (no transpose)
  -> y.T normalized via a PE-broadcast reciprocal -> projection emits
  out.T [C, T] partials. Matmuls run in float32r (single-pass fp32).

All T-length tensors are split into two half tiles (t<1024 / t>=1024) so the
scheduler can overlap half-1 qkv build with half-0 attention without false
dependencies; exp runs on [128, <=1024] score-pair tiles.
"""

import math
import numpy as np

import concourse.bass as bass
import concourse.mybir as mybir
from concourse import bacc, tile
from concourse.bass_utils import run_bass_kernel_spmd
from concourse.masks import make_identity

f32 = mybir.dt.float32
f32r = mybir.dt.float32r
i16 = mybir.dt.int16
i32 = mybir.dt.int32
AF = mybir.ActivationFunctionType
ALU = mybir.AluOpType

B, T, C, H, DH = 2, 2048, 1024, 16, 64
NCORES = 8
CPB = NCORES // B            # cores per batch (4)
HPC = H // CPB               # heads per core (4)
NHP = HPC // 2               # head pairs per core (2)
D_LOC = HPC * DH             # local feature dim (256)
NTB = T // 128               # 16 j-blocks
NTC = T // 512               # 4 t-chunks
HT = T // 2                  # half the sequence (1024)
SLOPES = [2.0 ** (-8.0 / H * (h + 1)) for h in range(H)]
INV_SQRT_D = 1.0 / math.sqrt(DH)


def build(nrep: int = 1, phases: str = 'full', dma_split: bool = True, bseg: int = 1024):
    nc = bacc.Bacc("TRN2", target_bir_lowering=False, debug=False)
    x_d = nc.dram_tensor("x", [T, C], f32r, kind="ExternalInput")
    wqkv_d = nc.dram_tensor("wqkvT", [C, 3 * D_LOC], f32r, kind="ExternalInput")
    bqkv_d = nc.dram_tensor("bqkv", [3 * D_LOC, 1], f32, kind="ExternalInput")
    wp_d = nc.dram_tensor("wpT", [D_LOC, C], f32r, kind="ExternalInput")
    bp_d = nc.dram_tensor("bp", [C, 1], f32, kind="ExternalInput")
    slopes_d = nc.dram_tensor("slopes", [HPC * 128, 1], f32, kind="ExternalInput")
    out_d = nc.dram_tensor("out_t", [C, T], f32, kind="ExternalOutput")

    with tile.TileContext(nc) as tc:
        with tc.tile_pool(name="const", bufs=1) as cp:
            ident_f = cp.tile([128, 128], f32)
            make_identity(nc, ident_f)
            ident_r = cp.tile([128, 128], f32r)
            nc.vector.tensor_copy(ident_r[:], ident_f[:])
            ones_f = cp.tile([1, 128], f32)
            nc.vector.memset(ones_f[:], 1.0)
            ones_row = cp.tile([1, 128], f32r)
            nc.vector.tensor_copy(ones_row[:], ones_f[:])
            ones_T = cp.tile([1, HT], f32)
            nc.vector.memset(ones_T[:], 1.0)
            ones_c8 = cp.tile([128, 8], f32)
            nc.vector.memset(ones_c8[:], 1.0)
            # -(t mod 1024) as int16 [1, HT]
            tneg = cp.tile([1, HT], i16)
            nc.gpsimd.iota(tneg[:], pattern=[[-1, HT]], base=0,
                           channel_multiplier=0)
            # slope scalars [1,1] and broadcast columns [128,1] per local head
            slope_sc, slope_bc = [], []
            for h in range(HPC):
                s = cp.tile([1, 1], f32, tag=f"slope{h}")
                nc.sync.dma_start(s[:], slopes_d.ap()[h * 128:h * 128 + 1, :])
                slope_sc.append(s)
                sb = cp.tile([128, 1], f32, tag=f"slopeb{h}")
                nc.sync.dma_start(sb[:], slopes_d.ap()[h * 128:(h + 1) * 128, :])
                slope_bc.append(sb)
            # ALiBi bias columns: slope_h*(jb*128 + jp - tcp*1024), per
            # (head, j-block, t-pair)
            bias_cols = {}
            for tcp in range(2):
                for jb in range(8 * (tcp + 1)):
                    icol = cp.tile([128, 1], i32, tag=f"i{jb}_{tcp}")
                    nc.gpsimd.iota(icol[:], pattern=[[0, 1]],
                                   base=jb * 128 - tcp * HT,
                                   channel_multiplier=1)
                    for h in range(HPC):
                        col = cp.tile([128, 1], f32, tag=f"b{h}_{jb}_{tcp}")
                        nc.scalar.activation(col[:], icol[:], AF.Copy,
                                             bias=0.0, scale=slope_bc[h][:])
                        bias_cols[(h, jb, tcp)] = col
            # qkv bias columns [128,1] per (type, head-pair); q's pre-scaled
            bqkv_cols = []
            for typ in range(3):
                row = []
                for hp in range(NHP):
                    colr = cp.tile([128, 1], f32, tag=f"bq{typ}_{hp}")
                    nc.sync.dma_start(
                        colr[:],
                        bqkv_d.ap()[typ * D_LOC + hp * 128:
                                    typ * D_LOC + (hp + 1) * 128, :])
                    if typ == 0:
                        cols = cp.tile([128, 1], f32, tag=f"bqs{hp}")
                        nc.scalar.mul(cols[:], colr[:], INV_SQRT_D)
                        row.append(cols)
                    else:
                        row.append(colr)
                bqkv_cols.append(row)
            bp_cols = cp.tile([128, 8], f32)
            for cc in range(8):
                nc.sync.dma_start(bp_cols[:, cc:cc + 1],
                                  bp_d.ap()[cc * 128:(cc + 1) * 128, :])

            def body(_iv=None):
                with tc.tile_pool(name="long", bufs=1) as lp:
                    # per-half long-lived tensors [t<1024, t>=1024]
                    qT = [[lp.tile([65, HT], f32r, name=f"qT{th}_{h}",
                                   tag=f"qT{th}_{h}") for h in range(HPC)]
                          for th in range(2)]
                    kT = [[lp.tile([65, HT], f32r, name=f"kT{th}_{h}",
                                   tag=f"kT{th}_{h}") for h in range(HPC)]
                          for th in range(2)]
                    vp = [[lp.tile([128, 8 * 65], f32r, name=f"vp{th}_{h}",
                                   tag=f"vp{th}_{h}") for h in range(HPC)]
                          for th in range(2)]
                    yT = [[lp.tile([128, HT], f32r, name=f"yT{th}_{hp}",
                                   tag=f"yT{th}_{hp}") for hp in range(NHP)]
                          for th in range(2)]
                    wpt = [lp.tile([128, C], f32r, name=f"wp{ds}", tag=f"wp{ds}")
                           for ds in range(NHP)]
                    for ds in range(NHP):
                        nc.sync.dma_start(wpt[ds][:],
                                          wp_d.ap()[ds * 128:(ds + 1) * 128, :])
                    for th in range(2):
                        for h in range(HPC):
                            # q' aug row: -slope * (t mod 1024)
                            nc.scalar.activation(qT[th][h][64:65, :], tneg[0:1, :],
                                                 AF.Copy, bias=0.0,
                                                 scale=slope_sc[h][0:1, 0:1])
                            nc.vector.tensor_copy(kT[th][h][64:65, :], ones_T[:])
                            nc.vector.tensor_copy(
                                vp[th][h].rearrange("p (j c) -> p j c", c=65)
                                [:, :, 64:65],
                                ones_c8[:].rearrange("p (j o) -> p j o", o=1))

                    def emit_build_half(th, xtp, xip, wqp, vtp, pst, psq):
                        xt = xtp.tile([128, 8 * 1024], f32r, name="xt", tag="xt")
                        xt3 = xt.rearrange("p (c t) -> p c t", t=1024)
                        vt = [vtp.tile([128, 1024], f32r, name=f"vt{hp}",
                                       tag=f"vt{hp}") for hp in range(NHP)]
                        for tcl in range(2):
                            for tbl in range(4):
                                tb = tcl * 4 + tbl
                                xin = xip.tile([128, C], f32r, tag="xin")
                                t0 = th * HT + tb * 128
                                nc.sync.dma_start(xin[:], x_d.ap()[t0:t0 + 128, :])
                                if phases == 'dma':
                                    yield
                                    continue
                                for g in range(2):
                                    ps = pst.tile([128, 512], f32r, tag="pst")
                                    for k4 in range(4):
                                        cc = g * 4 + k4
                                        nc.tensor.transpose(
                                            ps[:, k4 * 128:(k4 + 1) * 128],
                                            xin[:, cc * 128:(cc + 1) * 128],
                                            ident_r[:])
                                    nc.any.tensor_copy(
                                        xt3[:, g * 4:(g + 1) * 4,
                                            tb * 128:(tb + 1) * 128],
                                        ps.rearrange("p (c t) -> p c t", t=128))
                                yield
                            if phases == 'dma':
                                continue
                            for typ in range(3):
                                wt = [wqp.tile([128, D_LOC], f32r, name=f"w{cc}",
                                               tag=f"w{cc}") for cc in range(8)]
                                w_eng = nc.scalar if dma_split else nc.sync
                                for cc in range(8):
                                    w_eng.dma_start(
                                        wt[cc][:],
                                        wqkv_d.ap()[cc * 128:(cc + 1) * 128,
                                                    typ * D_LOC:(typ + 1) * D_LOC])
                                for hp in range(NHP):
                                    ps = psq.tile([128, 512], f32, tag="psq")
                                    for cc in range(8):
                                        nc.tensor.matmul(
                                            ps[:],
                                            wt[cc][:, hp * 128:(hp + 1) * 128],
                                            xt[:, cc * 1024 + tcl * 512:
                                               cc * 1024 + (tcl + 1) * 512],
                                            start=(cc == 0), stop=(cc == 7))
                                    tg = tcl * 512
                                    if typ < 2:
                                        dst = qT[th] if typ == 0 else kT[th]
                                        for sub in range(2):
                                            h = 2 * hp + sub
                                            bcol = bqkv_cols[typ][hp][
                                                sub * 64:(sub + 1) * 64, :]
                                            if typ == 0:
                                                nc.vector.tensor_scalar(
                                                    dst[h][0:64, tg:tg + 512],
                                                    ps[sub * 64:(sub + 1) * 64, :],
                                                    INV_SQRT_D, bcol,
                                                    ALU.mult, ALU.add)
                                            else:
                                                nc.vector.tensor_scalar_add(
                                                    dst[h][0:64, tg:tg + 512],
                                                    ps[sub * 64:(sub + 1) * 64, :],
                                                    bcol)
                                    else:
                                        nc.vector.tensor_scalar_add(
                                            vt[hp][:, tg:tg + 512],
                                            ps[:], bqkv_cols[2][hp][:])
                                    yield
                            # v' transposes for this quarter (j-blocks of tcl)
                            for h in range(HPC):
                                hp, sub = divmod(h, 2)
                                vp3 = vp[th][h].rearrange("p (j c) -> p j c", c=65)
                                ps = pst.tile([128, 256], f32r, tag="pst")
                                for k4 in range(4):
                                    jbl = tcl * 4 + k4
                                    nc.tensor.transpose(
                                        ps[:, k4 * 64:(k4 + 1) * 64],
                                        vt[hp][sub * 64:(sub + 1) * 64,
                                               jbl * 128:(jbl + 1) * 128],
                                        ident_r[sub * 64:(sub + 1) * 64,
                                                sub * 64:(sub + 1) * 64])
                                nc.any.tensor_copy(
                                    vp3[:, tcl * 4:(tcl + 1) * 4, 0:64],
                                    ps.rearrange("p (j c) -> p j c", c=64))
                                yield
                        if phases == 'dma':
                            return

                    def emit_attn(tcp, spool, ypool, seg):
                        for h in range(HPC):
                            hp, sub = divmod(h, 2)
                            y_acc = {}
                            for tcl in range(2):
                                y_acc[tcl] = ypool.tile([65, 512], f32,
                                                        name="y_acc", tag="y")

                            def emit_pv(jb, grp, pt):
                                vpt = vp[jb // 8][h]
                                jbl = jb % 8
                                for tc4 in grp:
                                    lo = (tc4 - grp[0]) * 512
                                    nc.tensor.matmul(
                                        y_acc[tc4 - 2 * tcp][:],
                                        vpt[:, jbl * 65:jbl * 65 + 65],
                                        pt[:, lo:lo + 512],
                                        start=(jb == 0),
                                        stop=(jb == 4 * tc4 + 3))

                            pending = []   # deferred PV: (jb, grp, pt)
                            for jb in range(8 * (tcp + 1)):
                                tc_lo = max(2 * tcp, jb // 4)
                                o = jb * 128 - tcp * HT
                                kTt = kT[jb // 8][h]
                                jbl = jb % 8
                                tcs = list(range(tc_lo, 2 * tcp + 2))
                                groups = [tcs] if seg == 1024 else [[t] for t in tcs]
                                for grp in groups:
                                    glo = (grp[0] - 2 * tcp) * 512
                                    gw = len(grp) * 512
                                    s = spool.tile([128, seg], f32, name="s",
                                                   tag="s")
                                    for tc4 in grp:
                                        lo = (tc4 - grp[0]) * 512
                                        nc.tensor.matmul(
                                            s[:, lo:lo + 512],
                                            kTt[:, jbl * 128:(jbl + 1) * 128],
                                            qT[tc4 // 2][h][:, (tc4 % 2) * 512:
                                                            (tc4 % 2) * 512 + 512],
                                            start=True, stop=True)
                                    pt = ptp.tile([128, seg], f32r, name="pt",
                                                  tag="pt")
                                    nc.scalar.activation(
                                        pt[:, 0:gw], s[:, 0:gw], AF.Exp,
                                        bias=bias_cols[(h, jb, tcp)][:], scale=1.0)
                                    if o + 128 > glo:
                                        w = min(o + 128, glo + gw) - glo
                                        nc.gpsimd.affine_select(
                                            out=pt[:, 0:w], in_=pt[:, 0:w],
                                            compare_op=ALU.is_ge, fill=0.0,
                                            base=glo - o,
                                            pattern=[[1, w]],
                                            channel_multiplier=-1)
                                    pending.append((jb, grp, pt))
                                    if len(pending) > 0:
                                        emit_pv(*pending.pop(0))
                                yield
                            for args in pending:
                                emit_pv(*args)
                            for tcl in range(2):
                                tc4 = 2 * tcp + tcl
                                rec = nrmp.tile([1, 512], f32r, name="rec",
                                                tag="rec")
                                with nc.allow_low_precision(
                                        reason="softmax denominator bcast"):
                                    nc.vector.reciprocal(
                                        rec[:], y_acc[tcl][64:65, :])
                                bc = spool.tile([128, 512], f32, name="bc",
                                                tag="s")
                                nc.tensor.matmul(bc[:], ones_row[:], rec[:],
                                                 start=True, stop=True)
                                bcs = nrmp.tile([128, 512], f32, name="bcs",
                                                tag="bcs")
                                nc.vector.tensor_copy(bcs[:], bc[:])
                                nc.vector.tensor_mul(
                                    yT[tcp][hp][sub * 64:(sub + 1) * 64,
                                                tcl * 512:(tcl + 1) * 512],
                                    y_acc[tcl][0:64, :], bcs[0:64, :])
                                yield

                    def emit_proj(tcp, ypool):
                        for cc in range(8):
                            for tcl in range(2):
                                tc4 = 2 * tcp + tcl
                                ps = ypool.tile([128, 512], f32, name="pp",
                                                tag="y")
                                for ds in range(NHP):
                                    nc.tensor.matmul(
                                        ps[:],
                                        wpt[ds][:, cc * 128:(cc + 1) * 128],
                                        yT[tcp][ds][:, tcl * 512:
                                                    (tcl + 1) * 512],
                                        start=(ds == 0), stop=(ds == NHP - 1))
                                ob = obp.tile([128, 512], f32, name="ob",
                                              tag="ob")
                                nc.vector.tensor_scalar_add(
                                    ob[:], ps[:], bp_cols[:, cc:cc + 1])
                                out_eng = nc.scalar if dma_split else nc.sync
                                out_eng.dma_start(
                                    out_d.ap()[cc * 128:(cc + 1) * 128,
                                               tc4 * 512:(tc4 + 1) * 512],
                                    ob[:])
                                yield

                    with tc.tile_pool(name="pt", bufs=4) as ptp, \
                         tc.tile_pool(name="nrm", bufs=2) as nrmp, \
                         tc.tile_pool(name="ob", bufs=3) as obp:
                        with tc.tile_pool(name="xt", bufs=1) as xtp, \
                             tc.tile_pool(name="xin", bufs=2) as xip, \
                             tc.tile_pool(name="w", bufs=1) as wqp, \
                             tc.tile_pool(name="vt", bufs=1) as vtp, \
                             tc.tile_pool(name="ps_t", bufs=1, space="PSUM") as pst, \
                             tc.tile_pool(name="ps_q", bufs=3, space="PSUM") as psq:
                            def drain(g):
                                for _ in g:
                                    pass

                            def interleave(ga, gb, ratio):
                                # emit ratio quanta of ga per quantum of gb
                                alive_a = alive_b = True
                                while alive_a or alive_b:
                                    for _ in range(ratio):
                                        if alive_a:
                                            try:
                                                next(ga)
                                            except StopIteration:
                                                alive_a = False
                                    if alive_b:
                                        try:
                                            next(gb)
                                        except StopIteration:
                                            alive_b = False

                            with tc.tile_pool(name="ps_sA", bufs=2,
                                              space="PSUM") as sA, \
                                 tc.tile_pool(name="ps_yA", bufs=2,
                                              space="PSUM") as yA:
                                drain(emit_build_half(0, xtp, xip, wqp, vtp,
                                                      pst, psq))
                                gb1 = emit_build_half(1, xtp, xip, wqp, vtp,
                                                      pst, psq)
                                if phases not in ('build', 'dma'):
                                    interleave(emit_attn(0, sA, yA, 512), gb1, 2)
                                else:
                                    drain(gb1)
                                if phases == 'full':
                                    drain(emit_proj(0, yA))
                        if phases not in ('build', 'dma'):
                            with tc.tile_pool(name="ps_sB", bufs=3,
                                              space="PSUM") as sB, \
                                 tc.tile_pool(name="ps_yB", bufs=2,
                                              space="PSUM") as yB:
                                gB = emit_attn(1, sB, yB, bseg)
                                if phases == 'full':
                                    drain(gB)
                                    drain(emit_proj(1, yB))
                                else:
                                    drain(gB)

            if phases in ('build', 'dma'):
                # dummy output write so the NEFF I/O signature matches
                zt = cp.tile([128, 512], f32, tag="zt")
                nc.vector.memset(zt[:], 0.0)
                for cc in range(8):
                    for tc4 in range(NTC):
                        nc.sync.dma_start(
                            out_d.ap()[cc * 128:(cc + 1) * 128,
                                       tc4 * 512:(tc4 + 1) * 512], zt[:])
            if nrep > 1:
                with tc.For_i(0, nrep, 1):
                    body()
            else:
                body()
    nc.compile()
    return nc


def shard_inputs(x, W_attn, b_attn, W_proj, b_proj):
    x = np.asarray(x, np.float32)
    W_attn = np.asarray(W_attn, np.float32)
    b_attn = np.asarray(b_attn, np.float32)
    W_proj = np.asarray(W_proj, np.float32)
    b_proj = np.asarray(b_proj, np.float32)
    in_maps = []
    for core in range(NCORES):
        b = core // CPB
        hg = core % CPB
        heads = list(range(hg * HPC, (hg + 1) * HPC))
        rows, brows = [], []
        for typ in range(3):
            for h in heads:
                r0 = typ * C + h * DH
                rows.append(W_attn[r0:r0 + DH])
                brows.append(b_attn[r0:r0 + DH])
        wqkvT = np.ascontiguousarray(np.concatenate(rows, 0).T)
        bqkv = np.concatenate(brows)[:, None].astype(np.float32)
        cols = np.concatenate([np.arange(h * DH, (h + 1) * DH) for h in heads])
        wpT = np.ascontiguousarray(W_proj[:, cols].T)
        bp = (b_proj[:, None] if hg == 0
              else np.zeros((C, 1))).astype(np.float32)
        slopes = np.repeat(np.array([SLOPES[h] for h in heads], np.float32),
                           128)[:, None]
        in_maps.append({
            "x": np.ascontiguousarray(x[b]),
            "wqkvT": wqkvT, "bqkv": bqkv, "wpT": wpT, "bp": bp,
            "slopes": slopes,
        })
    return in_maps


def unshard(results):
    y = np.zeros((B, T, C), np.float32)
    for core in range(NCORES):
        y[core // CPB] += results[core]["out_t"].T
    return y


_BUILD_CACHE = {}


def _built(nrep: int = 1):
    if nrep not in _BUILD_CACHE:
        _BUILD_CACHE[nrep] = build(nrep)
    return _BUILD_CACHE[nrep]


def kernel(**inputs) -> np.ndarray:
    in_maps = shard_inputs(inputs["x"], inputs["W_attn"], inputs["b_attn"],
                           inputs["W_proj"], inputs["b_proj"])
    nc = _built(1)
    res = run_bass_kernel_spmd(nc, in_maps, core_ids=list(range(NCORES)))
    return unshard(res.results)



# revision 3
# speedup vs baseline: 1.2258x; 1.2258x over previous
"""Self-contained Trainium2 Bass kernel: causal self-attention with ALiBi bias.

Reference computation (B=2, T=2048, C=1024, H=16, Dh=64):
    qkv = x @ W_attn.T + b_attn; split into q,k,v heads
    att = softmax(q.k/sqrt(Dh) + slope_h*min(c-r,0), causal)
    y = (att @ v, heads concat) @ W_proj.T + b_proj

Sharding (8 cores): 2-way data parallel on batch x 4-way tensor parallel on
heads (4 heads/core). Each core computes qkv for its heads over its batch,
full TxT attention for those heads, and the partial output projection over
its heads' 256 columns of W_proj; the host sums the 4 partials per batch.

Per-core dataflow keeps the feature dim on partitions ("transposed"):
  x.T via PE transposes -> qkv matmuls emit q.T/k.T/v.T -> scores computed
  as S.T[j,t] tiles (K=65 matmul; row 64 of q'.T carries -slope*(t%1024),
  row 64 of k'.T is ones) -> exp on ACT with per-partition ALiBi bias
  slope*(j - tpair0); the residual per-column exponent terms cancel in
  normalization (row 64 of v' is ones, giving the softmax denominator
  through the same PV matmul) -> P.T tiles (bf16) feed PV directly ->
  y.T normalized via Pool partition_broadcast of the reciprocal ->
  projection emits out.T [C, T] partials. Matmuls run in float32r except
  PV (bf16 P and v').

Engine/queue placement: wqkv is loaded ONCE into resident SBUF tiles on the
scalar (ACT) queue at startup along with all consts; x tiles stream on the
sync queue; out stores go on sync. ACT does only exp in steady state. The
output-projection for the first query half runs interleaved inside the
second attention window (PE has slack there; ACT is the limiter).
"""

import math
import numpy as np
import ml_dtypes

import concourse.bass as bass
import concourse.mybir as mybir
from concourse import bacc, tile
from concourse.bass_utils import run_bass_kernel_spmd
from concourse.masks import make_identity

f32 = mybir.dt.float32
f32r = mybir.dt.float32r
bf16 = mybir.dt.bfloat16
i16 = mybir.dt.int16
i32 = mybir.dt.int32
AF = mybir.ActivationFunctionType
ALU = mybir.AluOpType

B, T, C, H, DH = 2, 2048, 1024, 16, 64
NCORES = 8
CPB = NCORES // B            # cores per batch (4)
HPC = H // CPB               # heads per core (4)
NHP = HPC // 2               # head pairs per core (2)
D_LOC = HPC * DH             # local feature dim (256)
HT = T // 2                  # half the sequence (1024)
SLOPES = [2.0 ** (-8.0 / H * (h + 1)) for h in range(H)]
INV_SQRT_D = 1.0 / math.sqrt(DH)


def build(nrep: int = 1, phases: str = 'full'):
    nc = bacc.Bacc("TRN2", target_bir_lowering=False, debug=False)
    x_d = nc.dram_tensor("x", [T, C], bf16, kind="ExternalInput")
    wqkv_d = nc.dram_tensor("wqkvT", [C, 3 * D_LOC], bf16, kind="ExternalInput")
    # packed per-partition constants [128, 18]:
    #   cols 0:4   slope column per local head (slope_h repeated 128x)
    #   cols 4:10  qkv bias column per (typ, head-pair)
    #   cols 10:18 proj bias column per output C-chunk
    cst_d = nc.dram_tensor("cst", [128, 18], f32, kind="ExternalInput")
    wp_d = nc.dram_tensor("wpT", [D_LOC, C], f32r, kind="ExternalInput")
    out_d = nc.dram_tensor("out_t", [C, T], bf16, kind="ExternalOutput")

    with tile.TileContext(nc) as tc:
        with tc.tile_pool(name="const", bufs=1) as cp:
            ident_f = cp.tile([128, 128], f32)
            make_identity(nc, ident_f)
            ident_r = cp.tile([128, 128], f32r)
            nc.vector.tensor_copy(ident_r[:], ident_f[:])
            ones_T = cp.tile([1, HT], f32)
            nc.vector.memset(ones_T[:], 1.0)
            ones_c8 = cp.tile([128, 8], bf16)
            nc.vector.memset(ones_c8[:], 1.0)
            # -(t mod 1024) as int16 [1, HT]
            tneg = cp.tile([1, HT], i16)
            nc.gpsimd.iota(tneg[:], pattern=[[-1, HT]], base=0,
                           channel_multiplier=0)
            # one DMA for all small constants
            cst = cp.tile([128, 18], f32)
            nc.scalar.dma_start(cst[:], cst_d.ap()[:, :])
            slope_sc = [cst[0:1, h:h + 1] for h in range(HPC)]
            slope_bc = [cst[:, h:h + 1] for h in range(HPC)]
            bp_cols = cst[:, 10:18]
            # ALiBi bias columns: slope_h*(jb*128 + jp - tcp*1024), per
            # (head, j-block, t-pair)
            bias_cols = {}
            for tcp in range(2):
                for jb in range(8 * (tcp + 1)):
                    icol = cp.tile([128, 1], i32, tag=f"i{jb}_{tcp}")
                    nc.gpsimd.iota(icol[:], pattern=[[0, 1]],
                                   base=jb * 128 - tcp * HT,
                                   channel_multiplier=1)
                    for h in range(HPC):
                        col = cp.tile([128, 1], f32, tag=f"b{h}_{jb}_{tcp}")
                        nc.scalar.activation(col[:], icol[:], AF.Copy,
                                             bias=0.0, scale=slope_bc[h])
                        bias_cols[(h, jb, tcp)] = col
            # qkv bias columns [128,1] per (type, head-pair); q's pre-scaled
            bqkv_cols = []
            for typ in range(3):
                row = []
                for hp in range(NHP):
                    colr = cst[:, 4 + typ * NHP + hp:5 + typ * NHP + hp]
                    if typ == 0:
                        cols = cp.tile([128, 1], f32, tag=f"bqs{hp}")
                        nc.scalar.mul(cols[:], colr, INV_SQRT_D)
                        row.append(cols[:])
                    else:
                        row.append(colr)
                bqkv_cols.append(row)

            def body(_iv=None):
                with tc.tile_pool(name="long", bufs=1) as lp:
                    # per-half long-lived tensors [t<1024, t>=1024]
                    qT = [[lp.tile([65, HT], f32r, name=f"qT{th}_{h}",
                                   tag=f"qT{th}_{h}") for h in range(HPC)]
                          for th in range(2)]
                    kT = [[lp.tile([65, HT], f32r, name=f"kT{th}_{h}",
                                   tag=f"kT{th}_{h}") for h in range(HPC)]
                          for th in range(2)]
                    vp = [[lp.tile([128, 8 * 65], bf16, name=f"vp{th}_{h}",
                                   tag=f"vp{th}_{h}") for h in range(HPC)]
                          for th in range(2)]
                    yT = [[lp.tile([128, HT], f32r, name=f"yT{th}_{hp}",
                                   tag=f"yT{th}_{hp}") for hp in range(NHP)]
                          for th in range(2)]
                    # resident weights: wqkv as 8 C-chunks, wproj as 2 d-chunks
                    wq = [lp.tile([128, 3 * D_LOC], bf16, name=f"wq{cc}",
                                  tag=f"wq{cc}") for cc in range(8)]
                    for cc in range(8):
                        nc.scalar.dma_start(wq[cc][:],
                                            wqkv_d.ap()[cc * 128:(cc + 1) * 128, :])
                    wpt = [lp.tile([128, C], f32r, name=f"wp{ds}", tag=f"wp{ds}")
                           for ds in range(NHP)]
                    for ds in range(NHP):
                        nc.scalar.dma_start(wpt[ds][:],
                                            wp_d.ap()[ds * 128:(ds + 1) * 128, :])
                    # aug rows: q' row 64 = -slope*(t mod 1024) (same both
                    # halves: ACT once, Pool copy), k' row 64 = ones,
                    # v' row 64 = ones
                    for h in range(HPC):
                        nc.scalar.activation(qT[0][h][64:65, :], tneg[0:1, :],
                                             AF.Copy, bias=0.0,
                                             scale=slope_sc[h])
                        nc.gpsimd.tensor_copy(qT[1][h][64:65, :],
                                              qT[0][h][64:65, :])
                        for th in range(2):
                            nc.gpsimd.tensor_copy(kT[th][h][64:65, :], ones_T[:])
                            nc.gpsimd.tensor_copy(
                                vp[th][h].rearrange("p (j c) -> p j c", c=65)
                                [:, :, 64:65],
                                ones_c8[:].rearrange("p (j o) -> p j o", o=1))

                    def emit_build_half(th, xtp, xip, vtp, pst, psq):
                        xt = xtp.tile([128, 8 * 1024], bf16, name="xt", tag="xt")
                        xt3 = xt.rearrange("p (c t) -> p c t", t=1024)
                        vt = [vtp.tile([128, 1024], f32r, name=f"vt{hp}",
                                       tag=f"vt{hp}") for hp in range(NHP)]
                        # x.T via XBAR DMA-transpose straight into SBUF,
                        # quarter at a time for lower latency to first qkv:
                        # xt3[p, c, tcl*512+u] = x[th*HT + tcl*512 + u, c*128+p]
                        for tcl in range(2):
                            t0 = th * HT + tcl * 512
                            nc.sync.dma_start_transpose(
                                xt3[:, :, tcl * 512:(tcl + 1) * 512],
                                x_d.ap()[t0:t0 + 512, :])
                            yield
                        for typ in range(3):
                            for hp in range(NHP):
                                ps = psq.tile([128, 1024], f32, tag="psq")
                                for cc in range(8):
                                    for g in range(2):
                                        nc.tensor.matmul(
                                            ps[:, g * 512:(g + 1) * 512],
                                            wq[cc][:, typ * D_LOC + hp * 128:
                                                   typ * D_LOC + (hp + 1) * 128],
                                            xt[:, cc * 1024 + g * 512:
                                               cc * 1024 + (g + 1) * 512],
                                            start=(cc == 0), stop=(cc == 7))
                                if typ < 2:
                                    dst = qT[th] if typ == 0 else kT[th]
                                    for sub in range(2):
                                        h = 2 * hp + sub
                                        bcol = bqkv_cols[typ][hp][
                                            sub * 64:(sub + 1) * 64, :]
                                        if typ == 0:
                                            nc.vector.tensor_scalar(
                                                dst[h][0:64, :],
                                                ps[sub * 64:(sub + 1) * 64, :],
                                                INV_SQRT_D, bcol,
                                                ALU.mult, ALU.add)
                                        else:
                                            nc.vector.tensor_scalar_add(
                                                dst[h][0:64, :],
                                                ps[sub * 64:(sub + 1) * 64, :],
                                                bcol)
                                else:
                                    nc.vector.tensor_scalar_add(
                                        vt[hp][:], ps[:], bqkv_cols[2][hp])
                                yield
                        # v' transposes
                        for tcl in range(2):
                            for h in range(HPC):
                                hp, sub = divmod(h, 2)
                                vp3 = vp[th][h].rearrange("p (j c) -> p j c", c=65)
                                ps = pst.tile([128, 256], f32r, tag="pst")
                                for k4 in range(4):
                                    jbl = tcl * 4 + k4
                                    nc.tensor.transpose(
                                        ps[:, k4 * 64:(k4 + 1) * 64],
                                        vt[hp][sub * 64:(sub + 1) * 64,
                                               jbl * 128:(jbl + 1) * 128],
                                        ident_r[sub * 64:(sub + 1) * 64,
                                                sub * 64:(sub + 1) * 64])
                                nc.any.tensor_copy(
                                    vp3[:, tcl * 4:(tcl + 1) * 4, 0:64],
                                    ps.rearrange("p (j c) -> p j c", c=64))
                                yield

                    def norm_tc(h, y_acc, tc4, nrmp):
                        hp, sub = divmod(h, 2)
                        th, tcl = divmod(tc4, 2)
                        rec = nrmp.tile([1, 512], f32, name="rec", tag="rec")
                        nc.vector.reciprocal(rec[:], y_acc[64:65, :])
                        bcs = nrmp.tile([64, 512], f32, name="bcs", tag="bcs")
                        nc.gpsimd.partition_broadcast(bcs[:], rec[0:1, :],
                                                      channels=64)
                        nc.vector.tensor_mul(
                            yT[th][hp][sub * 64:(sub + 1) * 64,
                                       tcl * 512:(tcl + 1) * 512],
                            y_acc[0:64, :], bcs[:])

                    def emit_attn0(spool, ypool, ptp, nrmp):
                        # tc-outer: one y_acc alive at a time; 512-wide tiles
                        for h in range(HPC):
                            for tc4 in range(2):
                                nj = 4 * (tc4 + 1)
                                glo = tc4 * 512
                                y_acc = ypool.tile([65, 512], f32,
                                                   name="y_acc", tag="y")
                                pending = []

                                def pv0(jb, pt):
                                    nc.tensor.matmul(
                                        y_acc[:],
                                        vp[0][h][:, jb * 65:jb * 65 + 65],
                                        pt[:, 0:512],
                                        start=(jb == 0), stop=(jb == nj - 1))

                                for jb in range(nj):
                                    o = jb * 128
                                    # cols with t < j-block start are fully
                                    # masked: skip them in S and exp;
                                    # affine_select writes their zeros
                                    sk = min(max(0, o - glo), 512)
                                    s = spool.tile([128, 512], f32, name="s",
                                                   tag="s")
                                    nc.tensor.matmul(
                                        s[:, sk:512],
                                        kT[0][h][:, jb * 128:(jb + 1) * 128],
                                        qT[0][h][:, glo + sk:glo + 512],
                                        start=True, stop=True)
                                    pt = ptp.tile([128, 512], bf16, name="pt",
                                                  tag="pt")
                                    nc.scalar.activation(
                                        pt[:, sk:512], s[:, sk:512], AF.Exp,
                                        bias=bias_cols[(h, jb, 0)][:], scale=1.0)
                                    if o + 128 > glo:
                                        w = min(o + 128 - glo, 512)
                                        nc.gpsimd.affine_select(
                                            out=pt[:, 0:w], in_=pt[:, 0:w],
                                            compare_op=ALU.is_ge, fill=0.0,
                                            base=glo - o,
                                            pattern=[[1, w]],
                                            channel_multiplier=-1)
                                    pending.append((jb, pt))
                                    if len(pending) > 1:
                                        pv0(*pending.pop(0))
                                    yield
                                for args in pending:
                                    pv0(*args)
                                norm_tc(h, y_acc, tc4, nrmp)
                                yield

                    def emit_attn1(spool, ypool, ptp, nrmp):
                        # jb-outer with 1024-wide groups spanning tc2+tc3
                        tcp = 1
                        for h in range(HPC):
                            y_acc = {}
                            for tcl in range(2):
                                y_acc[tcl] = ypool.tile([65, 512], f32,
                                                        name="y_acc", tag="y")

                            def emit_pv(jb, grp, pt):
                                vpt = vp[jb // 8][h]
                                jbl = jb % 8
                                for tc4 in grp:
                                    lo = (tc4 - grp[0]) * 512
                                    nc.tensor.matmul(
                                        y_acc[tc4 - 2][:],
                                        vpt[:, jbl * 65:jbl * 65 + 65],
                                        pt[:, lo:lo + 512],
                                        start=(jb == 0),
                                        stop=(jb == 4 * tc4 + 3))

                            pending = []
                            for jb in range(16):
                                tc_lo = max(2, jb // 4)
                                o = jb * 128 - HT
                                kTt = kT[jb // 8][h]
                                jbl = jb % 8
                                grp = list(range(tc_lo, 4))
                                glo = (grp[0] - 2) * 512
                                gw = len(grp) * 512
                                sk = min(max(0, o - glo), gw)
                                s = spool.tile([128, 1024], f32, name="s",
                                               tag="s")
                                for tc4 in grp:
                                    lo = (tc4 - grp[0]) * 512
                                    csk = min(max(sk - lo, 0), 512)
                                    if csk < 512:
                                        nc.tensor.matmul(
                                            s[:, lo + csk:lo + 512],
                                            kTt[:, jbl * 128:(jbl + 1) * 128],
                                            qT[1][h][:, glo + lo + csk:
                                                     glo + lo + 512],
                                            start=True, stop=True)
                                pt = ptp.tile([128, 1024], bf16, name="pt",
                                              tag="pt")
                                nc.scalar.activation(
                                    pt[:, sk:gw], s[:, sk:gw], AF.Exp,
                                    bias=bias_cols[(h, jb, tcp)][:], scale=1.0)
                                if o + 128 > glo:
                                    w = min(o + 128, glo + gw) - glo
                                    nc.gpsimd.affine_select(
                                        out=pt[:, 0:w], in_=pt[:, 0:w],
                                        compare_op=ALU.is_ge, fill=0.0,
                                        base=glo - o,
                                        pattern=[[1, w]],
                                        channel_multiplier=-1)
                                pending.append((jb, grp, pt))
                                if len(pending) > 1:
                                    emit_pv(*pending.pop(0))
                                yield
                            for args in pending:
                                emit_pv(*args)
                            for tcl in range(2):
                                norm_tc(h, y_acc[tcl], 2 + tcl, nrmp)
                                yield

                    def emit_proj(tcp, pjp, obp):
                        # ds-outer within each cc so consecutive matmuls
                        # share the stationary wpt slice
                        for cc in range(8):
                            ps = [pjp.tile([128, 512], f32, name="pp",
                                           tag="pj") for _ in range(2)]
                            for ds in range(NHP):
                                for tcl in range(2):
                                    nc.tensor.matmul(
                                        ps[tcl][:],
                                        wpt[ds][:, cc * 128:(cc + 1) * 128],
                                        yT[tcp][ds][:, tcl * 512:
                                                    (tcl + 1) * 512],
                                        start=(ds == 0), stop=(ds == NHP - 1))
                            for tcl in range(2):
                                tc4 = 2 * tcp + tcl
                                ob = obp.tile([128, 512], bf16, name="ob",
                                              tag="ob")
                                if tcp == 1 and tcl == 0:
                                    # ACT is idle in the drain tail; split
                                    # the bias adds across ACT and DVE
                                    nc.scalar.activation(
                                        ob[:], ps[tcl][:], AF.Identity,
                                        bias=bp_cols[:, cc:cc + 1], scale=1.0)
                                else:
                                    nc.vector.tensor_scalar_add(
                                        ob[:], ps[tcl][:],
                                        bp_cols[:, cc:cc + 1])
                                nc.sync.dma_start(
                                    out_d.ap()[cc * 128:(cc + 1) * 128,
                                               tc4 * 512:(tc4 + 1) * 512],
                                    ob[:])
                                yield

                    def drain(g):
                        for _ in g:
                            pass

                    def interleave(ga, gb, ratio):
                        # emit ratio quanta of ga per quantum of gb
                        alive_a = alive_b = True
                        while alive_a or alive_b:
                            for _ in range(ratio):
                                if alive_a:
                                    try:
                                        next(ga)
                                    except StopIteration:
                                        alive_a = False
                            if alive_b:
                                try:
                                    next(gb)
                                except StopIteration:
                                    alive_b = False

                    with tc.tile_pool(name="pt", bufs=4) as ptp, \
                         tc.tile_pool(name="nrm", bufs=2) as nrmp, \
                         tc.tile_pool(name="ob", bufs=3) as obp:
                        with tc.tile_pool(name="xt", bufs=2) as xtp, \
                             tc.tile_pool(name="vt", bufs=2) as vtp, \
                             tc.tile_pool(name="ps_t", bufs=1,
                                          space="PSUM") as pst, \
                             tc.tile_pool(name="ps_q", bufs=2,
                                          space="PSUM") as psq:
                            with tc.tile_pool(name="ps_sA", bufs=2,
                                              space="PSUM") as sA, \
                                 tc.tile_pool(name="ps_yA", bufs=1,
                                              space="PSUM") as yA:
                                drain(emit_build_half(0, xtp, None, vtp,
                                                      pst, psq))
                                gb1 = emit_build_half(1, xtp, None, vtp,
                                                      pst, psq)
                                interleave(emit_attn0(sA, yA, ptp, nrmp),
                                           gb1, 3)
                        with tc.tile_pool(name="ps_sB", bufs=2,
                                          space="PSUM") as sB, \
                             tc.tile_pool(name="ps_yB", bufs=2,
                                          space="PSUM") as yB, \
                             tc.tile_pool(name="ps_pj", bufs=2,
                                          space="PSUM") as pjp:
                            interleave(emit_attn1(sB, yB, ptp, nrmp),
                                       emit_proj(0, pjp, obp), 4)
                            drain(emit_proj(1, pjp, obp))

            if nrep > 1:
                with tc.For_i(0, nrep, 1):
                    body()
            else:
                body()
    nc.compile()
    return nc


def shard_inputs(x, W_attn, b_attn, W_proj, b_proj):
    x = np.asarray(x, np.float32)
    W_attn = np.asarray(W_attn, np.float32)
    b_attn = np.asarray(b_attn, np.float32)
    W_proj = np.asarray(W_proj, np.float32)
    b_proj = np.asarray(b_proj, np.float32)
    in_maps = []
    for core in range(NCORES):
        b = core // CPB
        hg = core % CPB
        heads = list(range(hg * HPC, (hg + 1) * HPC))
        rows, brows = [], []
        for typ in range(3):
            for h in heads:
                r0 = typ * C + h * DH
                rows.append(W_attn[r0:r0 + DH])
                brows.append(b_attn[r0:r0 + DH])
        wqkvT = np.ascontiguousarray(
            np.concatenate(rows, 0).T).astype(ml_dtypes.bfloat16)
        bqkv = np.concatenate(brows).astype(np.float32)   # [3*D_LOC]
        cols = np.concatenate([np.arange(h * DH, (h + 1) * DH) for h in heads])
        wpT = np.ascontiguousarray(W_proj[:, cols].T)
        bp = (b_proj if hg == 0 else np.zeros(C)).astype(np.float32)
        # packed constants [128, 18]: slopes | qkv bias cols | proj bias cols
        cst = np.zeros((128, 18), np.float32)
        for hl in range(HPC):
            cst[:, hl] = SLOPES[heads[hl]]
        for typ in range(3):
            for hp in range(NHP):
                cst[:, 4 + typ * NHP + hp] = \
                    bqkv[typ * D_LOC + hp * 128:typ * D_LOC + (hp + 1) * 128]
        for cc in range(8):
            cst[:, 10 + cc] = bp[cc * 128:(cc + 1) * 128]
        in_maps.append({
            "x": np.ascontiguousarray(x[b]).astype(ml_dtypes.bfloat16),
            "wqkvT": wqkvT, "cst": cst, "wpT": wpT,
        })
    return in_maps


def unshard(results):
    y = np.zeros((B, T, C), np.float32)
    for core in range(NCORES):
        y[core // CPB] += results[core]["out_t"].astype(np.float32).T
    return y


_BUILD_CACHE = {}


def _built(nrep: int = 1):
    if nrep not in _BUILD_CACHE:
        _BUILD_CACHE[nrep] = build(nrep)
    return _BUILD_CACHE[nrep]


def kernel(**inputs) -> np.ndarray:
    in_maps = shard_inputs(inputs["x"], inputs["W_attn"], inputs["b_attn"],
                           inputs["W_proj"], inputs["b_proj"])
    nc = _built(1)
    res = run_bass_kernel_spmd(nc, in_maps, core_ids=list(range(NCORES)))
    return unshard(res.results)
